# revision 1
# baseline (speedup 1.0000x reference)
"""BERT embedding lookup on 8 TRN2 NeuronCores.

Strategy: data-parallel over batch. Core c handles batch rows [4c, 4c+4)
(2048 tokens). Per 128-token tile: indirect-DMA gather of word-embedding
rows, then one DVE op folds in position + token-type embeddings:
    out = word[id] + (posA[s] + tt * diff)
where posA = position_embedding + type_row0 and diff = type_row1 - type_row0
are precomputed on the host (cheap O(S*H) work), so the device does one
gather, two vector ops, one store per tile. No collectives.
"""

import numpy as np

P = 128
H = 768
VOCAB = 30522
SEQ = 512
BATCH = 32
N_CORES = 8
TOK_PER_CORE = BATCH * SEQ // N_CORES  # 2048
T_TILES = TOK_PER_CORE // P  # 16
S_BLOCKS = SEQ // P  # 4

_CACHE = {}


def _build(gather_bufs=6, out_bufs=6):
    from concourse import bacc, mybir
    import concourse.bass as bass
    import concourse.tile as tile

    nc = bacc.Bacc(
        "TRN2", target_bir_lowering=False, debug=False, num_devices=N_CORES
    )
    f32 = mybir.dt.float32
    i32 = mybir.dt.int32

    wemb = nc.dram_tensor("wemb", [VOCAB, H], f32, kind="ExternalInput").ap()
    posA = nc.dram_tensor("posA", [SEQ, H], f32, kind="ExternalInput").ap()
    diffr = nc.dram_tensor("diffr", [P, H], f32, kind="ExternalInput").ap()
    ids = nc.dram_tensor("ids", [P, T_TILES], i32, kind="ExternalInput").ap()
    ttf = nc.dram_tensor("ttf", [P, T_TILES], f32, kind="ExternalInput").ap()
    out = nc.dram_tensor("out", [TOK_PER_CORE, H], f32, kind="ExternalOutput").ap()

    with tile.TileContext(nc) as tc:
        with (
            tc.tile_pool(name="consts", bufs=1) as consts,
            tc.tile_pool(name="gather", bufs=gather_bufs) as gather_pool,
            tc.tile_pool(name="res", bufs=out_bufs) as res_pool,
        ):
            ids_sb = consts.tile([P, T_TILES], i32)
            nc.sync.dma_start(out=ids_sb[:], in_=ids[:])
            ttf_sb = consts.tile([P, T_TILES], f32)
            nc.sync.dma_start(out=ttf_sb[:], in_=ttf[:])
            diff_sb = consts.tile([P, H], f32)
            nc.sync.dma_start(out=diff_sb[:], in_=diffr[:])
            pos_sb = []
            for sb in range(S_BLOCKS):
                pt = consts.tile([P, H], f32, tag=f"pos{sb}")
                nc.sync.dma_start(out=pt[:], in_=posA[sb * P : (sb + 1) * P, :])
                pos_sb.append(pt)

            for t in range(T_TILES):
                sb = t % S_BLOCKS
                wt = gather_pool.tile([P, H], f32)
                nc.gpsimd.indirect_dma_start(
                    out=wt[:],
                    out_offset=None,
                    in_=wemb[:],
                    in_offset=bass.IndirectOffsetOnAxis(
                        ap=ids_sb[:, t : t + 1], axis=0
                    ),
                )
                res = res_pool.tile([P, H], f32)
                # res = diff * tt + posA[s-block]
                nc.vector.scalar_tensor_tensor(
                    out=res[:],
                    in0=diff_sb[:],
                    scalar=ttf_sb[:, t : t + 1],
                    in1=pos_sb[sb][:],
                    op0=mybir.AluOpType.mult,
                    op1=mybir.AluOpType.add,
                )
                nc.vector.tensor_add(out=res[:], in0=res[:], in1=wt[:])
                nc.sync.dma_start(out=out[t * P : (t + 1) * P, :], in_=res[:])

    nc.compile()
    return nc


def _get_nc():
    if "nc" not in _CACHE:
        _CACHE["nc"] = _build()
    return _CACHE["nc"]


def kernel(
    input_ids, token_type_ids, word_embedding, position_embedding, token_type_embedding
):
    from concourse.bass_utils import run_bass_kernel_spmd

    nc = _get_nc()

    ids = np.ascontiguousarray(
        np.asarray(input_ids, dtype=np.int32)
        .reshape(N_CORES, T_TILES, P)
        .transpose(0, 2, 1)
    )
    ttf = np.ascontiguousarray(
        np.asarray(token_type_ids, dtype=np.float32)
        .reshape(N_CORES, T_TILES, P)
        .transpose(0, 2, 1)
    )
    wemb = np.ascontiguousarray(np.asarray(word_embedding, dtype=np.float32))
    pos = np.asarray(position_embedding, dtype=np.float32)
    typ = np.asarray(token_type_embedding, dtype=np.float32)
    posA = np.ascontiguousarray(pos + typ[0][None, :])
    diffr = np.ascontiguousarray(np.broadcast_to(typ[1] - typ[0], (P, H)))

    in_maps = [
        {"wemb": wemb, "posA": posA, "diffr": diffr, "ids": ids[c], "ttf": ttf[c]}
        for c in range(N_CORES)
    ]
    r = run_bass_kernel_spmd(nc, in_maps, core_ids=list(range(N_CORES)))
    out = np.stack([r.results[c]["out"] for c in range(N_CORES)], axis=0)
    return out.reshape(BATCH, SEQ, H)
